# revision 42
# baseline (speedup 1.0000x reference)
"""Trainium2 Bass kernel for nn_PairwiseConv (gnn_message_passing).

Reference computation, for each edge e=(i,j) of a sparse adjacency:
    pair[b,o,e] = sum_c W[o,c,0]*x[b,c,i] + W[o,c,1]*x[b,c,j] + bias[o]
    y[b,o,n]    = (sum_{e: i_e=n} pair[b,o,e]) / max(deg_j[n],1)
    y[b,127,n]  = deg_j[n]            (counts channel)
where deg_j[n] = #{e: j_e = n}.

Algebraic reformulation (exact):
    y[b,o,n] = S[b,o,n]*recip[n] + (W0^T x)[b,o,n]*c1[n] + bias[o]*c1[n]
    S        = W1^T (x @ AT),  AT[m,n] = #{e: j_e=m, i_e=n}
    recip[n] = 1/max(deg_j[n],1),  c1[n] = deg_i[n]*recip[n]
Key trick: contract x against the count matrix FIRST (P = x @ AT), then
apply the 128x128 conv weights to the much smaller P. The only heavy
matmul is P, done in fp8(e4m3) DoubleRow mode (256-row contraction per
pass) -- counts are small ints (exact in fp8) and the fp8 error on x
only touches the minority S term of the output.

Sharding: 8 cores x 512 dst-node slices, all 4 batches per core.

DMA is the wall (the two HWDGE queues share ~230GB/s and crawl for the
first few us), so most of AT is NOT shipped: pairs 4..15 are built
on-chip by gpsimd local_scatter from host-packed (cell,index) tables --
the scatter writes int16 cells that alias two adjacent fp8 AT entries
(bitcast view). Only pairs 0..3 (needed before the scatters spin up)
ride the DMA. x^T pair-blocks are interleaved across both hardware
queues; a dozen dummy fp8 matmuls warm the PE's p-state during the DMA
lead-in. All outputs leave as bf16 in a single wide DMA.
"""

import numpy as np
import ml_dtypes

import concourse.bass as bass
import concourse.mybir as mybir
import concourse.tile as tile
from concourse import bacc
from concourse.bass_utils import run_bass_kernel_spmd

B = 4
C = 128   # in channels
O = 128   # out channels incl. counts row (127 real + zero row)
N = 4096
SLICE = 512   # dst nodes per core
NCORES = 8
MC = 32       # 128-row source chunks
KP = MC // 2  # chunk pairs (DoubleRow)
F32 = mybir.dt.float32
BF16 = mybir.dt.bfloat16
F8 = mybir.dt.float8e4
I16 = mybir.dt.int16
BF16_NP = ml_dtypes.bfloat16
F8_NP = ml_dtypes.float8_e4m3
DR = mybir.MatmulPerfMode.DoubleRow

AT_DMA_PAIRS = 4          # leading AT pairs shipped by DMA
NSCAT = KP - AT_DMA_PAIRS  # trailing pairs built by gpsimd local_scatter
NWARM = 12                 # PE warm-up matmuls
# x^T pair-blocks, interleaved across the two hardware queues
XT_SCALAR = [(0, 2), (4, 6), (8, 10), (12, 14)]
XT_SYNC = [(2, 4), (6, 8), (10, 12), (14, 16)]


def _pack_cells(at8):
    """Locate nonzero int16 cells (pairs of adjacent fp8 entries) of the
    scatter-built AT pairs."""
    cells = np.ascontiguousarray(at8[:, AT_DMA_PAIRS:]).view(np.uint16)
    cells = cells.reshape(128, NSCAT, SLICE)
    p, k, u = np.nonzero(cells)
    v = cells[p, k, u]
    order = np.lexsort((u, p, k))
    p, k, u, v = p[order], k[order], u[order], v[order]
    gid = k * 128 + p
    percell = np.bincount(gid, minlength=NSCAT * 128)
    return gid, u, v, percell


def prep_inputs(x, W, b, idx_i, idx_j):
    x = np.ascontiguousarray(np.asarray(x, np.float32))
    W = np.asarray(W, np.float32)
    bias = np.asarray(b, np.float32)
    ii = np.asarray(idx_i).astype(np.int64)
    jj = np.asarray(idx_j).astype(np.int64)

    # x^T pair-major [p=m%128, (kp, b, t, c)] -- same for all cores
    # xt8[p, k, b, t, c] = x[b, c, (2k+t)*128 + p]
    xt8 = np.ascontiguousarray(
        x.transpose(2, 0, 1)              # [N, B, C]
        .reshape(KP, 2, 128, B, C)        # [k, t, p, b, c]
        .transpose(2, 0, 3, 1, 4)         # [p, k, b, t, c]
        .reshape(128, KP * B, 2, C)
    ).astype(F8_NP)

    # conv weights as lhsT [c, o], o=127 padded with a zero column;
    # both kernels packed in one tensor (one DMA, 512B rows)
    W01 = np.zeros((128, 256), BF16_NP)
    W01[:, :127] = W[:, :, 0].T.astype(BF16_NP)
    W01[:, 128:255] = W[:, :, 1].T.astype(BF16_NP)

    deg_j = np.bincount(jj, minlength=N).astype(np.float32)
    deg_i = np.bincount(ii, minlength=N).astype(np.float32)
    maxdj = np.maximum(deg_j, 1.0)
    recip = (1.0 / maxdj).astype(np.float32)
    c1 = (deg_i / maxdj).astype(np.float32)

    # fold the bias into the W0 pass: solve W0 @ delta = bias so that
    # W0^T (x + delta) = W0^T x + bias exactly (W0 is 127x128, full rank)
    delta = np.linalg.lstsq(W[:, :, 0], bias, rcond=None)[0]

    packed = []
    ni = 2
    for s in range(NCORES):
        base = s * SLICE
        sel = (ii >= base) & (ii < base + SLICE)
        atf = np.zeros((N, SLICE), np.float32)
        np.add.at(atf, (jj[sel], ii[sel] - base), 1.0)
        at8 = np.ascontiguousarray(
            atf.reshape(KP, 2, 128, SLICE).transpose(2, 0, 1, 3)
            .reshape(128, KP, 2, SLICE)
        ).astype(F8_NP)
        gid, u, v, percell = _pack_cells(at8)
        ni = max(ni, int(percell.max()))
        packed.append((at8, gid, u, v, percell))
    ni += ni % 2  # local_scatter wants an even num_idxs

    in_maps = []
    for s in range(NCORES):
        base = s * SLICE
        at8, gid, u, v, percell = packed[s]
        idxs = np.full((NSCAT * 128, ni), -1, np.int16)
        vals = np.zeros((NSCAT * 128, ni), np.uint16)
        pos = np.arange(len(gid)) - np.concatenate(
            ([0], np.cumsum(percell)))[gid]
        idxs[gid, pos] = u.astype(np.int16)
        vals[gid, pos] = v
        # TAB layout [128, 2, NSCAT, ni]: [:,0]=idxs, [:,1]=vals
        tab = np.empty((128, 2, NSCAT, ni), np.int16)
        tab[:, 0] = idxs.reshape(NSCAT, 128, ni).transpose(1, 0, 2)
        tab[:, 1] = vals.view(np.int16).reshape(
            NSCAT, 128, ni).transpose(1, 0, 2)

        rs = recip[base:base + SLICE]
        c1s = c1[base:base + SLICE]
        # bf16 x slice, bias-shifted and pre-scaled by c1 (covers the
        # deg_i*(W0^T x) + bias*c1 terms in one matmul pass)
        xs = np.ascontiguousarray(
            ((x[:, :, base:base + SLICE] + delta[None, :, None])
             * c1s[None, None, :])
            .transpose(1, 0, 2).reshape(128, B * SLICE)
        ).astype(BF16_NP)
        recipf = np.ascontiguousarray(
            np.broadcast_to(rs[None, :], (128, SLICE))).astype(BF16_NP)
        in_maps.append({
            "XT8": xt8,
            "AT8H": np.ascontiguousarray(at8[:, :AT_DMA_PAIRS]),
            "TAB": np.ascontiguousarray(tab),
            "XS": xs, "W01": W01, "RECIPF": recipf,
        })
    return in_maps, ni, deg_j


def build_program(ni):
    nc = bacc.Bacc("TRN2", target_bir_lowering=False, debug=False,
                   num_devices=NCORES)

    XT8 = nc.dram_tensor("XT8", [128, KP * B, 2, C], F8, kind="ExternalInput")
    AT8H = nc.dram_tensor("AT8H", [128, AT_DMA_PAIRS, 2, SLICE], F8,
                          kind="ExternalInput")
    TAB = nc.dram_tensor("TAB", [128, 2, NSCAT, ni], I16,
                         kind="ExternalInput")
    XS = nc.dram_tensor("XS", [128, B * SLICE], BF16, kind="ExternalInput")
    W01 = nc.dram_tensor("W01", [128, 256], BF16, kind="ExternalInput")
    RECIPF = nc.dram_tensor("RECIPF", [128, SLICE], BF16,
                            kind="ExternalInput")
    YOUT = nc.dram_tensor("YOUT", [O, B * SLICE], BF16,
                          kind="ExternalOutput")

    with tile.TileContext(nc) as tc:
        with (
            tc.tile_pool(name="big", bufs=1) as bigp,
            tc.tile_pool(name="const", bufs=1) as constp,
            tc.tile_pool(name="ps_P", bufs=1, space="PSUM") as ps_P_p,
            tc.tile_pool(name="ps_y", bufs=3, space="PSUM") as ps_y_p,
            tc.tile_pool(name="ps_w", bufs=1, space="PSUM") as ps_w_p,
        ):
            psbp = constp
            ostp = constp
            at = bigp.tile([128, KP, 2, SLICE], F8)
            xt = bigp.tile([128, KP * B, 2, C], F8)
            tabt = constp.tile([128, 2, NSCAT, ni], I16)

            # sync queue: first AT pair, scatter tables, rest of the DMA'd
            # AT pairs, x^T share
            nc.sync.dma_start(at[:, 0:2, :, :], AT8H[:, 0:2, :, :])
            nc.sync.dma_start(tabt[:], TAB[:])
            nc.sync.dma_start(at[:, 2:AT_DMA_PAIRS, :, :],
                              AT8H[:, 2:AT_DMA_PAIRS, :, :])
            # scalar queue: x^T (interleaved with sync's share)
            for q in range(len(XT_SCALAR)):
                lo, hi = XT_SCALAR[q]
                nc.scalar.dma_start(xt[:, lo * B:hi * B, :, :],
                                    XT8[:, lo * B:hi * B, :, :])
                lo, hi = XT_SYNC[q]
                nc.sync.dma_start(xt[:, lo * B:hi * B, :, :],
                                  XT8[:, lo * B:hi * B, :, :])
            xs = constp.tile([128, B * SLICE], BF16)
            nc.scalar.dma_start(xs[:], XS[:])
            recipf = constp.tile([128, SLICE], BF16)
            nc.scalar.dma_start(recipf[:], RECIPF[:])
            w01 = constp.tile([128, 256], BF16)
            nc.scalar.dma_start(w01[:], W01[:])
            w0t = w01[:, 0:128]
            w1t = w01[:, 128:256]

            # gpsimd: build trailing AT pairs from the tables (SBUF-local)
            for k in range(NSCAT):
                dst = at[:, AT_DMA_PAIRS + k, :, :].bitcast(I16)
                nc.gpsimd.local_scatter(
                    out_ap=dst.rearrange("p a b -> p (a b)"),
                    data_ap=tabt[:, 1, k, :],
                    idxs_ap=tabt[:, 0, k, :],
                    channels=128, num_elems=SLICE, num_idxs=ni,
                )

            # PE warm-up on memset tiles while DMA streams in (p-state ramp)
            wlhs = constp.tile([128, 2, C], F8)
            nc.vector.memset(wlhs[:], 0.0)
            wrhs = constp.tile([128, 2, SLICE], F8)
            nc.vector.memset(wrhs[:], 0.0)
            ps_w = ps_w_p.tile([128, SLICE], F32, tag="warm", name="ps_warm")
            for _ in range(NWARM):
                nc.tensor.matmul(ps_w[:], wlhs[:], wrhs[:],
                                 start=True, stop=True,
                                 perf_mode=DR, skip_group_check=True)

            # phase P: ps_P[b] += xT[pair k, b]^T @ AT[pair k] (fp8 DoubleRow)
            ps_Ps = [ps_P_p.tile([128, SLICE], F32, tag=f"pp{bi}",
                                 name=f"ps_P{bi}") for bi in range(B)]

            def pmm(k, bi):
                nc.tensor.matmul(
                    ps_Ps[bi][:],
                    xt[:, k * B + bi, :, :],
                    at[:, k, :, :],
                    start=(k == 0), stop=(k == KP - 1),
                    perf_mode=DR, skip_group_check=True,
                )

            for k in range(KP - 1):
                for bi in range(B):
                    pmm(k, bi)
            ost = ostp.tile([O, B * SLICE], BF16, tag="ost", name="ost")
            # last pair batch-by-batch with each batch's epilogue chained
            # in right behind its final accumulation
            for bi in range(B):
                pmm(KP - 1, bi)
                psb = psbp.tile([128, SLICE], BF16, tag=f"psb{bi}",
                                name=f"psb{bi}")
                nc.vector.tensor_mul(psb[:], ps_Ps[bi][:], recipf[:])
                ps_y = ps_y_p.tile([128, SLICE], F32, tag="py",
                                   name=f"ps_y{bi}")
                nc.tensor.matmul(ps_y[:], w1t, psb[:],
                                 start=True, stop=False, skip_group_check=True)
                nc.tensor.matmul(ps_y[:], w0t,
                                 xs[:, bi * SLICE:(bi + 1) * SLICE],
                                 start=False, stop=True, skip_group_check=True)
                nc.scalar.copy(ost[:, bi * SLICE:(bi + 1) * SLICE],
                               ps_y[:])
                if bi == 1:
                    nc.scalar.dma_start(YOUT[:, :2 * SLICE],
                                        ost[:, :2 * SLICE])
            nc.scalar.dma_start(YOUT[:, 2 * SLICE:], ost[:, 2 * SLICE:])

    nc.compile()
    return nc


def kernel(x, W, b, idx_i, idx_j):
    in_maps, ni, deg_j = prep_inputs(x, W, b, idx_i, idx_j)
    nc = build_program(ni)
    res = run_bass_kernel_spmd(nc, in_maps, list(range(NCORES)))
    y = np.empty((B, O, N), np.float32)
    for s in range(NCORES):
        yc = res.results[s]["YOUT"].astype(np.float32)
        for bi in range(B):
            y[bi, :, s * SLICE:(s + 1) * SLICE] = \
                yc[:, bi * SLICE:(bi + 1) * SLICE]
    # counts channel is a pure function of idx_j: write it exactly on host
    y[:, 127, :] = deg_j[None, :]
    return y


if __name__ == "__main__":
    rng = np.random.default_rng(0)
    x = rng.standard_normal((B, C, N), np.float32)
    W = rng.standard_normal((127, C, 2), np.float32) * 0.05
    b = rng.standard_normal((127,), np.float32) * 0.05
    idx_i = rng.integers(0, N, 131072)
    idx_j = rng.integers(0, N, 131072)
    y = kernel(x, W, b, idx_i, idx_j)
    print("ok", y.shape, float(np.abs(y).mean()))


# revision 43
# speedup vs baseline: 1.0938x; 1.0938x over previous
"""Trainium2 Bass kernel for nn_PairwiseConv (gnn_message_passing).

Reference computation, for each edge e=(i,j) of a sparse adjacency:
    pair[b,o,e] = sum_c W[o,c,0]*x[b,c,i] + W[o,c,1]*x[b,c,j] + bias[o]
    y[b,o,n]    = (sum_{e: i_e=n} pair[b,o,e]) / max(deg_j[n],1)
    y[b,127,n]  = deg_j[n]            (counts channel)
where deg_j[n] = #{e: j_e = n}.

Algebraic reformulation (exact):
    y[b,o,n] = S[b,o,n]*recip[n] + (W0^T x)[b,o,n]*c1[n] + bias[o]*c1[n]
    S        = W1^T (x @ AT),  AT[m,n] = #{e: j_e=m, i_e=n}
    recip[n] = 1/max(deg_j[n],1),  c1[n] = deg_i[n]*recip[n]
Key trick: contract x against the count matrix FIRST (P = x @ AT), then
apply the 128x128 conv weights to the much smaller P. The only heavy
matmul is P, done in fp8(e4m3) DoubleRow mode (256-row contraction per
pass) -- counts are small ints (exact in fp8) and the fp8 error on x
only touches the minority S term of the output.

Sharding: 8 cores x 512 dst-node slices, all 4 batches per core.

DMA is the wall (the two HWDGE queues share ~230GB/s and crawl for the
first few us), so most of AT is NOT shipped: pairs 4..15 are built
on-chip by gpsimd local_scatter from host-packed (cell,index) tables --
the scatter writes int16 cells that alias two adjacent fp8 AT entries
(bitcast view). Only pairs 0..3 (needed before the scatters spin up)
ride the DMA. x^T pair-blocks are interleaved across both hardware
queues; a dozen dummy fp8 matmuls warm the PE's p-state during the DMA
lead-in. All outputs leave as bf16 in a single wide DMA.
"""

import numpy as np
import ml_dtypes

import concourse.bass as bass
import concourse.mybir as mybir
import concourse.tile as tile
from concourse import bacc
from concourse.bass_utils import run_bass_kernel_spmd

B = 4
C = 128   # in channels
O = 128   # out channels incl. counts row (127 real + zero row)
N = 4096
SLICE = 512   # dst nodes per core
NCORES = 8
MC = 32       # 128-row source chunks
KP = MC // 2  # chunk pairs (DoubleRow)
F32 = mybir.dt.float32
BF16 = mybir.dt.bfloat16
F8 = mybir.dt.float8e4
I16 = mybir.dt.int16
BF16_NP = ml_dtypes.bfloat16
F8_NP = ml_dtypes.float8_e4m3
DR = mybir.MatmulPerfMode.DoubleRow

AT_DMA_PAIRS = 4          # leading AT pairs shipped by DMA
NSCAT = KP - AT_DMA_PAIRS  # trailing pairs built by gpsimd local_scatter
NWARM = 12                 # PE warm-up matmuls
# x^T pair-blocks, interleaved across the two hardware queues
XT_SCALAR = [(0, 2), (4, 6), (8, 10), (12, 14)]
XT_SYNC = [(2, 4), (6, 8), (10, 12), (14, 16)]


def _pack_cells(at8):
    """Locate nonzero int16 cells (pairs of adjacent fp8 entries) of the
    scatter-built AT pairs."""
    cells = np.ascontiguousarray(at8[:, AT_DMA_PAIRS:]).view(np.uint16)
    cells = cells.reshape(128, NSCAT, SLICE)
    p, k, u = np.nonzero(cells)
    v = cells[p, k, u]
    order = np.lexsort((u, p, k))
    p, k, u, v = p[order], k[order], u[order], v[order]
    gid = k * 128 + p
    percell = np.bincount(gid, minlength=NSCAT * 128)
    return gid, u, v, percell


def prep_inputs(x, W, b, idx_i, idx_j):
    x = np.ascontiguousarray(np.asarray(x, np.float32))
    W = np.asarray(W, np.float32)
    bias = np.asarray(b, np.float32)
    ii = np.asarray(idx_i).astype(np.int64)
    jj = np.asarray(idx_j).astype(np.int64)

    # x^T pair-major [p=m%128, (kp, b, t, c)] -- same for all cores
    # xt8[p, k, b, t, c] = x[b, c, (2k+t)*128 + p]
    xt8 = np.ascontiguousarray(
        x.transpose(2, 0, 1)              # [N, B, C]
        .reshape(KP, 2, 128, B, C)        # [k, t, p, b, c]
        .transpose(2, 0, 3, 1, 4)         # [p, k, b, t, c]
        .reshape(128, KP * B, 2, C)
    ).astype(F8_NP)

    # conv weights as lhsT [c, o], o=127 padded with a zero column;
    # both kernels packed in one tensor (one DMA, 512B rows)
    W01 = np.zeros((128, 256), BF16_NP)
    W01[:, :127] = W[:, :, 0].T.astype(BF16_NP)
    W01[:, 128:255] = W[:, :, 1].T.astype(BF16_NP)

    deg_j = np.bincount(jj, minlength=N).astype(np.float32)
    deg_i = np.bincount(ii, minlength=N).astype(np.float32)
    maxdj = np.maximum(deg_j, 1.0)
    recip = (1.0 / maxdj).astype(np.float32)
    c1 = (deg_i / maxdj).astype(np.float32)

    # fold the bias into the W0 pass: solve W0 @ delta = bias so that
    # W0^T (x + delta) = W0^T x + bias exactly (W0 is 127x128, full rank)
    delta = np.linalg.lstsq(W[:, :, 0], bias, rcond=None)[0]

    packed = []
    ni = 2
    for s in range(NCORES):
        base = s * SLICE
        sel = (ii >= base) & (ii < base + SLICE)
        atf = np.zeros((N, SLICE), np.float32)
        np.add.at(atf, (jj[sel], ii[sel] - base), 1.0)
        at8 = np.ascontiguousarray(
            atf.reshape(KP, 2, 128, SLICE).transpose(2, 0, 1, 3)
            .reshape(128, KP, 2, SLICE)
        ).astype(F8_NP)
        gid, u, v, percell = _pack_cells(at8)
        ni = max(ni, int(percell.max()))
        packed.append((at8, gid, u, v, percell))
    ni += ni % 2  # local_scatter wants an even num_idxs

    in_maps = []
    for s in range(NCORES):
        base = s * SLICE
        at8, gid, u, v, percell = packed[s]
        idxs = np.full((NSCAT * 128, ni), -1, np.int16)
        vals = np.zeros((NSCAT * 128, ni), np.uint16)
        pos = np.arange(len(gid)) - np.concatenate(
            ([0], np.cumsum(percell)))[gid]
        idxs[gid, pos] = u.astype(np.int16)
        vals[gid, pos] = v
        # TAB layout [128, 2, NSCAT, ni]: [:,0]=idxs, [:,1]=vals
        tab = np.empty((128, 2, NSCAT, ni), np.int16)
        tab[:, 0] = idxs.reshape(NSCAT, 128, ni).transpose(1, 0, 2)
        tab[:, 1] = vals.view(np.int16).reshape(
            NSCAT, 128, ni).transpose(1, 0, 2)

        rs = recip[base:base + SLICE]
        c1s = c1[base:base + SLICE]
        # bf16 x slice, bias-shifted and pre-scaled by c1 (covers the
        # deg_i*(W0^T x) + bias*c1 terms in one matmul pass)
        xs = np.ascontiguousarray(
            ((x[:, :, base:base + SLICE] + delta[None, :, None])
             * c1s[None, None, :])
            .transpose(1, 0, 2).reshape(128, B * SLICE)
        ).astype(BF16_NP)
        recipf = np.ascontiguousarray(
            np.broadcast_to(rs[None, :], (128, SLICE))).astype(BF16_NP)
        in_maps.append({
            "XT8": xt8,
            "AT8H": np.ascontiguousarray(at8[:, :AT_DMA_PAIRS]),
            "TAB": np.ascontiguousarray(tab),
            "XS": xs, "W01": W01, "RECIPF": recipf,
        })
    return in_maps, ni, deg_j


def build_program(ni):
    nc = bacc.Bacc("TRN2", target_bir_lowering=False, debug=False,
                   num_devices=NCORES)

    XT8 = nc.dram_tensor("XT8", [128, KP * B, 2, C], F8, kind="ExternalInput")
    AT8H = nc.dram_tensor("AT8H", [128, AT_DMA_PAIRS, 2, SLICE], F8,
                          kind="ExternalInput")
    TAB = nc.dram_tensor("TAB", [128, 2, NSCAT, ni], I16,
                         kind="ExternalInput")
    XS = nc.dram_tensor("XS", [128, B * SLICE], BF16, kind="ExternalInput")
    W01 = nc.dram_tensor("W01", [128, 256], BF16, kind="ExternalInput")
    RECIPF = nc.dram_tensor("RECIPF", [128, SLICE], BF16,
                            kind="ExternalInput")
    YOUT = nc.dram_tensor("YOUT", [O, B * SLICE], BF16,
                          kind="ExternalOutput")

    with tile.TileContext(nc) as tc:
        with (
            tc.tile_pool(name="big", bufs=1) as bigp,
            tc.tile_pool(name="const", bufs=1) as constp,
            tc.tile_pool(name="ps_P", bufs=1, space="PSUM") as ps_P_p,
            tc.tile_pool(name="ps_y", bufs=3, space="PSUM") as ps_y_p,
            tc.tile_pool(name="ps_w", bufs=1, space="PSUM") as ps_w_p,
        ):
            psbp = constp
            ostp = constp
            at = bigp.tile([128, KP, 2, SLICE], F8)
            xt = bigp.tile([128, KP * B, 2, C], F8)
            tabt = constp.tile([128, 2, NSCAT, ni], I16)

            # sync queue: first AT pair, scatter tables, rest of the DMA'd
            # AT pairs, x^T share
            nc.sync.dma_start(at[:, 0:2, :, :], AT8H[:, 0:2, :, :])
            nc.sync.dma_start(at[:, 2:AT_DMA_PAIRS, :, :],
                              AT8H[:, 2:AT_DMA_PAIRS, :, :])
            # scalar queue: x^T (interleaved with sync's share); scatter
            # tables follow the first block so sync's AT head lands early
            for q in range(len(XT_SCALAR)):
                lo, hi = XT_SCALAR[q]
                nc.scalar.dma_start(xt[:, lo * B:hi * B, :, :],
                                    XT8[:, lo * B:hi * B, :, :])
                if q == 0:
                    nc.scalar.dma_start(tabt[:], TAB[:])
                lo, hi = XT_SYNC[q]
                nc.sync.dma_start(xt[:, lo * B:hi * B, :, :],
                                  XT8[:, lo * B:hi * B, :, :])
            xs = constp.tile([128, B * SLICE], BF16)
            nc.scalar.dma_start(xs[:], XS[:])
            recipf = constp.tile([128, SLICE], BF16)
            nc.scalar.dma_start(recipf[:], RECIPF[:])
            w01 = constp.tile([128, 256], BF16)
            nc.scalar.dma_start(w01[:], W01[:])
            w0t = w01[:, 0:128]
            w1t = w01[:, 128:256]

            # gpsimd: build trailing AT pairs from the tables (SBUF-local)
            for k in range(NSCAT):
                dst = at[:, AT_DMA_PAIRS + k, :, :].bitcast(I16)
                nc.gpsimd.local_scatter(
                    out_ap=dst.rearrange("p a b -> p (a b)"),
                    data_ap=tabt[:, 1, k, :],
                    idxs_ap=tabt[:, 0, k, :],
                    channels=128, num_elems=SLICE, num_idxs=ni,
                )

            # PE warm-up on memset tiles while DMA streams in (p-state ramp)
            wlhs = constp.tile([128, 2, C], F8)
            nc.vector.memset(wlhs[:], 0.0)
            wrhs = constp.tile([128, 2, SLICE], F8)
            nc.vector.memset(wrhs[:], 0.0)
            ps_w = ps_w_p.tile([128, SLICE], F32, tag="warm", name="ps_warm")
            for _ in range(NWARM):
                nc.tensor.matmul(ps_w[:], wlhs[:], wrhs[:],
                                 start=True, stop=True,
                                 perf_mode=DR, skip_group_check=True)

            # phase P: ps_P[b] += xT[pair k, b]^T @ AT[pair k] (fp8 DoubleRow)
            ps_Ps = [ps_P_p.tile([128, SLICE], F32, tag=f"pp{bi}",
                                 name=f"ps_P{bi}") for bi in range(B)]

            def pmm(k, bi):
                nc.tensor.matmul(
                    ps_Ps[bi][:],
                    xt[:, k * B + bi, :, :],
                    at[:, k, :, :],
                    start=(k == 0), stop=(k == KP - 1),
                    perf_mode=DR, skip_group_check=True,
                )

            for k in range(KP - 1):
                for bi in range(B):
                    pmm(k, bi)
            ost = ostp.tile([O, B * SLICE], BF16, tag="ost", name="ost")
            # last pair batch-by-batch with each batch's epilogue chained
            # in right behind its final accumulation
            for bi in range(B):
                pmm(KP - 1, bi)
                psb = psbp.tile([128, SLICE], BF16, tag=f"psb{bi}",
                                name=f"psb{bi}")
                nc.vector.tensor_mul(psb[:], ps_Ps[bi][:], recipf[:])
                ps_y = ps_y_p.tile([128, SLICE], F32, tag="py",
                                   name=f"ps_y{bi}")
                nc.tensor.matmul(ps_y[:], w1t, psb[:],
                                 start=True, stop=False, skip_group_check=True)
                nc.tensor.matmul(ps_y[:], w0t,
                                 xs[:, bi * SLICE:(bi + 1) * SLICE],
                                 start=False, stop=True, skip_group_check=True)
                nc.scalar.copy(ost[:, bi * SLICE:(bi + 1) * SLICE],
                               ps_y[:])
                if bi == 1:
                    nc.scalar.dma_start(YOUT[:, :2 * SLICE],
                                        ost[:, :2 * SLICE])
            nc.scalar.dma_start(YOUT[:, 2 * SLICE:], ost[:, 2 * SLICE:])

    nc.compile()
    return nc


def kernel(x, W, b, idx_i, idx_j):
    in_maps, ni, deg_j = prep_inputs(x, W, b, idx_i, idx_j)
    nc = build_program(ni)
    res = run_bass_kernel_spmd(nc, in_maps, list(range(NCORES)))
    y = np.empty((B, O, N), np.float32)
    for s in range(NCORES):
        yc = res.results[s]["YOUT"].astype(np.float32)
        for bi in range(B):
            y[bi, :, s * SLICE:(s + 1) * SLICE] = \
                yc[:, bi * SLICE:(bi + 1) * SLICE]
    # counts channel is a pure function of idx_j: write it exactly on host
    y[:, 127, :] = deg_j[None, :]
    return y


if __name__ == "__main__":
    rng = np.random.default_rng(0)
    x = rng.standard_normal((B, C, N), np.float32)
    W = rng.standard_normal((127, C, 2), np.float32) * 0.05
    b = rng.standard_normal((127,), np.float32) * 0.05
    idx_i = rng.integers(0, N, 131072)
    idx_j = rng.integers(0, N, 131072)
    y = kernel(x, W, b, idx_i, idx_j)
    print("ok", y.shape, float(np.abs(y).mean()))


# revision 44
# speedup vs baseline: 1.1060x; 1.0111x over previous
"""Trainium2 Bass kernel for nn_PairwiseConv (gnn_message_passing).

Reference computation, for each edge e=(i,j) of a sparse adjacency:
    pair[b,o,e] = sum_c W[o,c,0]*x[b,c,i] + W[o,c,1]*x[b,c,j] + bias[o]
    y[b,o,n]    = (sum_{e: i_e=n} pair[b,o,e]) / max(deg_j[n],1)
    y[b,127,n]  = deg_j[n]            (counts channel)
where deg_j[n] = #{e: j_e = n}.

Algebraic reformulation (exact):
    y[b,o,n] = S[b,o,n]*recip[n] + (W0^T x)[b,o,n]*c1[n] + bias[o]*c1[n]
    S        = W1^T (x @ AT),  AT[m,n] = #{e: j_e=m, i_e=n}
    recip[n] = 1/max(deg_j[n],1),  c1[n] = deg_i[n]*recip[n]
Key trick: contract x against the count matrix FIRST (P = x @ AT), then
apply the 128x128 conv weights to the much smaller P. The only heavy
matmul is P, done in fp8(e4m3) DoubleRow mode (256-row contraction per
pass) -- counts are small ints (exact in fp8) and the fp8 error on x
only touches the minority S term of the output.

Sharding: 8 cores x 512 dst-node slices, all 4 batches per core.

DMA is the wall (the two HWDGE queues share ~230GB/s and crawl for the
first few us), so most of AT is NOT shipped: pairs 4..15 are built
on-chip by gpsimd local_scatter from host-packed (cell,index) tables --
the scatter writes int16 cells that alias two adjacent fp8 AT entries
(bitcast view). Only pairs 0..3 (needed before the scatters spin up)
ride the DMA. x^T pair-blocks are interleaved across both hardware
queues; a dozen dummy fp8 matmuls warm the PE's p-state during the DMA
lead-in. All outputs leave as bf16 in a single wide DMA.
"""

import numpy as np
import ml_dtypes

import concourse.bass as bass
import concourse.mybir as mybir
import concourse.tile as tile
from concourse import bacc
from concourse.bass_utils import run_bass_kernel_spmd

B = 4
C = 128   # in channels
O = 128   # out channels incl. counts row (127 real + zero row)
N = 4096
SLICE = 512   # dst nodes per core
NCORES = 8
MC = 32       # 128-row source chunks
KP = MC // 2  # chunk pairs (DoubleRow)
F32 = mybir.dt.float32
BF16 = mybir.dt.bfloat16
F8 = mybir.dt.float8e4
I16 = mybir.dt.int16
BF16_NP = ml_dtypes.bfloat16
F8_NP = ml_dtypes.float8_e4m3
DR = mybir.MatmulPerfMode.DoubleRow

AT_DMA_PAIRS = 4          # leading AT pairs shipped by DMA
NSCAT = KP - AT_DMA_PAIRS  # trailing pairs built by gpsimd local_scatter
NWARM = 10                 # PE warm-up matmuls
# x^T pair-blocks, interleaved across the two hardware queues
XT_SCALAR = [(0, 2), (4, 6), (8, 10), (12, 14)]
XT_SYNC = [(2, 4), (6, 8), (10, 12), (14, 16)]


def _pack_cells(at8):
    """Locate nonzero int16 cells (pairs of adjacent fp8 entries) of the
    scatter-built AT pairs."""
    cells = np.ascontiguousarray(at8[:, AT_DMA_PAIRS:]).view(np.uint16)
    cells = cells.reshape(128, NSCAT, SLICE)
    p, k, u = np.nonzero(cells)
    v = cells[p, k, u]
    order = np.lexsort((u, p, k))
    p, k, u, v = p[order], k[order], u[order], v[order]
    gid = k * 128 + p
    percell = np.bincount(gid, minlength=NSCAT * 128)
    return gid, u, v, percell


def prep_inputs(x, W, b, idx_i, idx_j):
    x = np.ascontiguousarray(np.asarray(x, np.float32))
    W = np.asarray(W, np.float32)
    bias = np.asarray(b, np.float32)
    ii = np.asarray(idx_i).astype(np.int64)
    jj = np.asarray(idx_j).astype(np.int64)

    # x^T pair-major [p=m%128, (kp, b, t, c)] -- same for all cores
    # xt8[p, k, b, t, c] = x[b, c, (2k+t)*128 + p]
    xt8 = np.ascontiguousarray(
        x.transpose(2, 0, 1)              # [N, B, C]
        .reshape(KP, 2, 128, B, C)        # [k, t, p, b, c]
        .transpose(2, 0, 3, 1, 4)         # [p, k, b, t, c]
        .reshape(128, KP * B, 2, C)
    ).astype(F8_NP)

    # conv weights as lhsT [c, o], o=127 padded with a zero column;
    # both kernels packed in one tensor (one DMA, 512B rows)
    W01 = np.zeros((128, 256), BF16_NP)
    W01[:, :127] = W[:, :, 0].T.astype(BF16_NP)
    W01[:, 128:255] = W[:, :, 1].T.astype(BF16_NP)

    deg_j = np.bincount(jj, minlength=N).astype(np.float32)
    deg_i = np.bincount(ii, minlength=N).astype(np.float32)
    maxdj = np.maximum(deg_j, 1.0)
    recip = (1.0 / maxdj).astype(np.float32)
    c1 = (deg_i / maxdj).astype(np.float32)

    # fold the bias into the W0 pass: solve W0 @ delta = bias so that
    # W0^T (x + delta) = W0^T x + bias exactly (W0 is 127x128, full rank)
    delta = np.linalg.lstsq(W[:, :, 0], bias, rcond=None)[0]

    packed = []
    ni = 2
    for s in range(NCORES):
        base = s * SLICE
        sel = (ii >= base) & (ii < base + SLICE)
        atf = np.zeros((N, SLICE), np.float32)
        np.add.at(atf, (jj[sel], ii[sel] - base), 1.0)
        at8 = np.ascontiguousarray(
            atf.reshape(KP, 2, 128, SLICE).transpose(2, 0, 1, 3)
            .reshape(128, KP, 2, SLICE)
        ).astype(F8_NP)
        gid, u, v, percell = _pack_cells(at8)
        ni = max(ni, int(percell.max()))
        packed.append((at8, gid, u, v, percell))
    ni += ni % 2  # local_scatter wants an even num_idxs

    in_maps = []
    for s in range(NCORES):
        base = s * SLICE
        at8, gid, u, v, percell = packed[s]
        idxs = np.full((NSCAT * 128, ni), -1, np.int16)
        vals = np.zeros((NSCAT * 128, ni), np.uint16)
        pos = np.arange(len(gid)) - np.concatenate(
            ([0], np.cumsum(percell)))[gid]
        idxs[gid, pos] = u.astype(np.int16)
        vals[gid, pos] = v
        # TAB layout [128, 2, NSCAT, ni]: [:,0]=idxs, [:,1]=vals
        tab = np.empty((128, 2, NSCAT, ni), np.int16)
        tab[:, 0] = idxs.reshape(NSCAT, 128, ni).transpose(1, 0, 2)
        tab[:, 1] = vals.view(np.int16).reshape(
            NSCAT, 128, ni).transpose(1, 0, 2)

        rs = recip[base:base + SLICE]
        c1s = c1[base:base + SLICE]
        # bf16 x slice, bias-shifted and pre-scaled by c1 (covers the
        # deg_i*(W0^T x) + bias*c1 terms in one matmul pass)
        xs = np.ascontiguousarray(
            ((x[:, :, base:base + SLICE] + delta[None, :, None])
             * c1s[None, None, :])
            .transpose(1, 0, 2).reshape(128, B * SLICE)
        ).astype(BF16_NP)
        recipf = np.ascontiguousarray(
            np.broadcast_to(rs[None, :], (128, SLICE))).astype(BF16_NP)
        in_maps.append({
            "XT8": xt8,
            "AT8H": np.ascontiguousarray(at8[:, :AT_DMA_PAIRS]),
            "TAB": np.ascontiguousarray(tab),
            "XS": xs, "W01": W01, "RECIPF": recipf,
        })
    return in_maps, ni, deg_j


def build_program(ni):
    nc = bacc.Bacc("TRN2", target_bir_lowering=False, debug=False,
                   num_devices=NCORES)

    XT8 = nc.dram_tensor("XT8", [128, KP * B, 2, C], F8, kind="ExternalInput")
    AT8H = nc.dram_tensor("AT8H", [128, AT_DMA_PAIRS, 2, SLICE], F8,
                          kind="ExternalInput")
    TAB = nc.dram_tensor("TAB", [128, 2, NSCAT, ni], I16,
                         kind="ExternalInput")
    XS = nc.dram_tensor("XS", [128, B * SLICE], BF16, kind="ExternalInput")
    W01 = nc.dram_tensor("W01", [128, 256], BF16, kind="ExternalInput")
    RECIPF = nc.dram_tensor("RECIPF", [128, SLICE], BF16,
                            kind="ExternalInput")
    YOUT = nc.dram_tensor("YOUT", [O, B * SLICE], BF16,
                          kind="ExternalOutput")

    with tile.TileContext(nc) as tc:
        with (
            tc.tile_pool(name="big", bufs=1) as bigp,
            tc.tile_pool(name="const", bufs=1) as constp,
            tc.tile_pool(name="ps_P", bufs=1, space="PSUM") as ps_P_p,
            tc.tile_pool(name="ps_y", bufs=3, space="PSUM") as ps_y_p,
            tc.tile_pool(name="ps_w", bufs=1, space="PSUM") as ps_w_p,
        ):
            psbp = constp
            ostp = constp
            at = bigp.tile([128, KP, 2, SLICE], F8)
            xt = bigp.tile([128, KP * B, 2, C], F8)
            tabt = constp.tile([128, 2, NSCAT, ni], I16)

            # sync queue: first AT pair, scatter tables, rest of the DMA'd
            # AT pairs, x^T share
            nc.sync.dma_start(at[:, 0:2, :, :], AT8H[:, 0:2, :, :])
            nc.sync.dma_start(at[:, 2:AT_DMA_PAIRS, :, :],
                              AT8H[:, 2:AT_DMA_PAIRS, :, :])
            # scalar queue: x^T (interleaved with sync's share); scatter
            # tables follow the first block so sync's AT head lands early
            for q in range(len(XT_SCALAR)):
                lo, hi = XT_SCALAR[q]
                nc.scalar.dma_start(xt[:, lo * B:hi * B, :, :],
                                    XT8[:, lo * B:hi * B, :, :])
                if q == 0:
                    nc.scalar.dma_start(tabt[:], TAB[:])
                lo, hi = XT_SYNC[q]
                nc.sync.dma_start(xt[:, lo * B:hi * B, :, :],
                                  XT8[:, lo * B:hi * B, :, :])
            xs = constp.tile([128, B * SLICE], BF16)
            nc.scalar.dma_start(xs[:], XS[:])
            recipf = constp.tile([128, SLICE], BF16)
            nc.scalar.dma_start(recipf[:], RECIPF[:])
            w01 = constp.tile([128, 256], BF16)
            nc.scalar.dma_start(w01[:], W01[:])
            w0t = w01[:, 0:128]
            w1t = w01[:, 128:256]

            # gpsimd: build trailing AT pairs from the tables (SBUF-local)
            for k in range(NSCAT):
                dst = at[:, AT_DMA_PAIRS + k, :, :].bitcast(I16)
                nc.gpsimd.local_scatter(
                    out_ap=dst.rearrange("p a b -> p (a b)"),
                    data_ap=tabt[:, 1, k, :],
                    idxs_ap=tabt[:, 0, k, :],
                    channels=128, num_elems=SLICE, num_idxs=ni,
                )

            # PE warm-up on memset tiles while DMA streams in (p-state ramp)
            wlhs = constp.tile([128, 2, C], F8)
            nc.vector.memset(wlhs[:], 0.0)
            wrhs = constp.tile([128, 2, SLICE], F8)
            nc.vector.memset(wrhs[:], 0.0)
            ps_w = ps_w_p.tile([128, SLICE], F32, tag="warm", name="ps_warm")
            for _ in range(NWARM):
                nc.tensor.matmul(ps_w[:], wlhs[:], wrhs[:],
                                 start=True, stop=True,
                                 perf_mode=DR, skip_group_check=True)

            # phase P: ps_P[b] += xT[pair k, b]^T @ AT[pair k] (fp8 DoubleRow)
            ps_Ps = [ps_P_p.tile([128, SLICE], F32, tag=f"pp{bi}",
                                 name=f"ps_P{bi}") for bi in range(B)]

            def pmm(k, bi):
                nc.tensor.matmul(
                    ps_Ps[bi][:],
                    xt[:, k * B + bi, :, :],
                    at[:, k, :, :],
                    start=(k == 0), stop=(k == KP - 1),
                    perf_mode=DR, skip_group_check=True,
                )

            for k in range(KP - 1):
                for bi in range(B):
                    pmm(k, bi)
            ost = ostp.tile([O, B * SLICE], BF16, tag="ost", name="ost")
            # last pair batch-by-batch with each batch's epilogue chained
            # in right behind its final accumulation
            for bi in range(B):
                pmm(KP - 1, bi)
                psb = psbp.tile([128, SLICE], BF16, tag=f"psb{bi}",
                                name=f"psb{bi}")
                nc.vector.tensor_mul(psb[:], ps_Ps[bi][:], recipf[:])
                ps_y = ps_y_p.tile([128, SLICE], F32, tag="py",
                                   name=f"ps_y{bi}")
                nc.tensor.matmul(ps_y[:], w1t, psb[:],
                                 start=True, stop=False, skip_group_check=True)
                nc.tensor.matmul(ps_y[:], w0t,
                                 xs[:, bi * SLICE:(bi + 1) * SLICE],
                                 start=False, stop=True, skip_group_check=True)
                nc.scalar.copy(ost[:, bi * SLICE:(bi + 1) * SLICE],
                               ps_y[:])
                if bi == 1:
                    nc.scalar.dma_start(YOUT[:, :2 * SLICE],
                                        ost[:, :2 * SLICE])
            nc.scalar.dma_start(YOUT[:, 2 * SLICE:], ost[:, 2 * SLICE:])

    nc.compile()
    return nc


def kernel(x, W, b, idx_i, idx_j):
    in_maps, ni, deg_j = prep_inputs(x, W, b, idx_i, idx_j)
    nc = build_program(ni)
    res = run_bass_kernel_spmd(nc, in_maps, list(range(NCORES)))
    y = np.empty((B, O, N), np.float32)
    for s in range(NCORES):
        yc = res.results[s]["YOUT"].astype(np.float32)
        for bi in range(B):
            y[bi, :, s * SLICE:(s + 1) * SLICE] = \
                yc[:, bi * SLICE:(bi + 1) * SLICE]
    # counts channel is a pure function of idx_j: write it exactly on host
    y[:, 127, :] = deg_j[None, :]
    return y


if __name__ == "__main__":
    rng = np.random.default_rng(0)
    x = rng.standard_normal((B, C, N), np.float32)
    W = rng.standard_normal((127, C, 2), np.float32) * 0.05
    b = rng.standard_normal((127,), np.float32) * 0.05
    idx_i = rng.integers(0, N, 131072)
    idx_j = rng.integers(0, N, 131072)
    y = kernel(x, W, b, idx_i, idx_j)
    print("ok", y.shape, float(np.abs(y).mean()))
